# revision 1
# baseline (speedup 1.0000x reference)
"""Trainium2 Bass kernel for nn_AttentionBlock (GroupNorm + single-head
self-attention over 64x64 spatial + out-projection + residual).

Sharding: 8 cores = 4 batches x 2 query-halves. Each core receives its
batch's x as [512, 4096] (channels x pixels), rotated so that its own
2048 query pixels are columns 0:2048. GroupNorm stats / keys / values
span all 4096 pixels (invariant to the rotation), so the program is
identical on every core (pure SPMD, no collectives); the host gathers
the 8 [512, 2048] outputs back into (4, 512, 64, 64).

Algebraic restructuring (host-precomputed):
  - scores^T = h^T (M h + c0),  M = Wk^T Wq, c0 = Wk^T bq: k is never
    materialized (the k-bias term is constant within each softmax
    column and cancels exactly).
  - v^T = (Wv h)^T with no bias; bv commutes through the attention
    average and folds into bo2 = out_w @ bv + out_b.
  - softmax without max-subtraction (scores are O(5); exp is safe in
    fp32). The denominator D is accumulated on the DVE from the exp
    tiles and summed across partitions with one ones-matmul per chunk;
    since 1/D is per-query it commutes with the out-projection, so the
    reciprocal runs off the PE critical path.

Precision: weights / h / u / E / v^T / att in bf16 (PE multiplies at
FP22, accumulates fp32 in PSUM); GroupNorm statistics in fp32 on a
bf16 copy of x; the residual uses the original fp32 x. Measured
relative error vs the fp32 reference: ~4.5e-4.

Per core the PE executes ~1270 matmuls of [128x128]x[128x512]
(~24 GFLOP at ~213 ns each when warm). Exec time ~356 us/core in the
chip's fast clock state.

Infrastructure workarounds (this container's walrus accepts at most
one sync-wait per instruction): Tile's kernel-tail drain waits are
re-emitted as single-wait NOPs, and a post-scheduling pass hoists
extra waits from any instruction onto preceding single-wait NOPs.
"""

import numpy as np
import ml_dtypes

import concourse.bass as bass
import concourse.bass_isa as bass_isa
from concourse import library_config
import concourse.mybir as mybir
import concourse.tile as tile
from concourse.tile_scheduler import N_PROCS
from concourse.vector_clock import ScopedClock, VectorClock

F32 = mybir.dt.float32
F32R = mybir.dt.float32r
BT = mybir.dt.bfloat16
AF = mybir.ActivationFunctionType
OP = mybir.AluOpType

PART = 128
C = 512          # channels
N = 4096         # pixels per batch
NQ = 2048        # query pixels per core
CT = C // PART   # 4 channel tiles
NKT = N // PART  # 32 key tiles
CH = 512         # nq chunk width
JCH = NQ // CH   # 4 chunks
EPS = 1e-5
SCALE = float(C) ** -0.5


def _patched_drain_and_barrier(self, tick_clock, wait_clock):
    # Walrus in this container accepts at most one sync-wait per
    # instruction; Tile's stock exit path stacks every outstanding
    # proc's wait on a single SP Drain. Emit one single-wait NOP per
    # proc instead, then a wait-free drain.
    nc = self.nc
    gc = tick_clock.global_clock
    for p in range(N_PROCS):
        t = gc[p]
        if t <= 0:
            continue
        vc = VectorClock([t if q == p else 0 for q in range(N_PROCS)])
        nop = nc.sync.nop(nofuse=True, hint=f"drainwait{p}")
        wait_clock.add_sem_waits(nop.ins, ScopedClock({None: vc}))
    nc.sync.drain()

    nc.all_engine_barrier()
    assert self.sems is not None
    popped = nc._tile_sem_poison_stack.pop()
    assert popped is self._sem_poison
    nc.clear_and_free_semaphores(list(self.sems.allocated().values()))


def apply_tile_patch():
    tile.TileContext._drain_and_barrier = _patched_drain_and_barrier


def split_multi_waits(nc):
    """Walrus in this container accepts at most one sync-wait command per
    instruction. Tile's wait-assignment freely stacks several. Hoist all
    but the last wait of each instruction onto single-wait NOPs inserted
    immediately before it on the same engine (engine blocks on each in
    turn, so the gating is equivalent)."""
    k = 0
    for fn in nc.m.functions:
        for bb in fn.blocks:
            il = bb.instructions
            i = 0
            while i < len(il):
                inst = il[i]
                si = inst.sync_info
                waits = list(si.on_wait) if si and si.on_wait else []
                if len(waits) > 1:
                    for w in waits[:-1]:
                        nop = mybir.InstNoOp(name=f"I-waitsplit-{k}")
                        k += 1
                        nop.engine = inst.engine
                        nop.sync_info = mybir.SyncInfo(on_wait=[w], on_update=[])
                        il.insert(i, nop)
                        i += 1
                    si.on_wait = [waits[-1]]
                    inst.sync_info = si
                i += 1


STATS_MODE = 'halves'


def build_program(split_waits=True, stats_mode=None):
    stats_mode = stats_mode or STATS_MODE
    apply_tile_patch()
    nc = bass.Bass(name="attnblk")
    xa = nc.dram_tensor("xa", [C, N], F32, kind="ExternalInput").ap()
    xb = nc.dram_tensor("xb", [C, N], BT, kind="ExternalInput").ap()
    mt = nc.dram_tensor("mt", [C, C], BT, kind="ExternalInput").ap()
    wvt = nc.dram_tensor("wvt", [C, C], BT, kind="ExternalInput").ap()
    wot = nc.dram_tensor("wot", [C, C], BT, kind="ExternalInput").ap()
    gw = nc.dram_tensor("gw", [PART, CT], F32, kind="ExternalInput").ap()
    gb = nc.dram_tensor("gb", [PART, CT], F32, kind="ExternalInput").ap()
    c0t = nc.dram_tensor("c0t", [PART, CT], F32, kind="ExternalInput").ap()
    bo2t = nc.dram_tensor("bo2t", [PART, CT], F32, kind="ExternalInput").ap()
    gmat = nc.dram_tensor("gmat", [PART, 8], F32R, kind="ExternalInput").ap()
    gmatt = nc.dram_tensor("gmatt", [8, PART], F32R, kind="ExternalInput").ap()
    onesd = nc.dram_tensor("onesd", [PART, PART], BT, kind="ExternalInput").ap()
    onesfd = nc.dram_tensor("onesfd", [PART, PART], F32R, kind="ExternalInput").ap()
    y = nc.dram_tensor("y", [C, NQ], F32, kind="ExternalOutput").ap()

    with tile.TileContext(nc) as tc:
        with (
            tc.tile_pool(name="const", bufs=1) as cp,
            tc.tile_pool(name="wts", bufs=1) as wp,
            tc.tile_pool(name="hp", bufs=1) as hp,
        ):
            gwt = cp.tile([PART, CT], F32)
            nc.sync.dma_start(out=gwt, in_=gw)
            gbt = cp.tile([PART, CT], F32)
            nc.sync.dma_start(out=gbt, in_=gb)
            c0s = cp.tile([PART, CT], F32)
            nc.sync.dma_start(out=c0s, in_=c0t)
            bo2s = cp.tile([PART, CT], F32)
            nc.sync.dma_start(out=bo2s, in_=bo2t)
            gm = cp.tile([PART, 8], F32R)
            nc.sync.dma_start(out=gm, in_=gmat)
            gmt = cp.tile([8, PART], F32R)
            nc.sync.dma_start(out=gmt, in_=gmatt)
            ones = cp.tile([PART, PART], BT)
            onesf = cp.tile([PART, PART], F32R)
            epst = cp.tile([PART, 1], F32)
            nc.vector.memset(epst, EPS)

            mts = wp.tile([PART, CT, C], BT)
            wvts = wp.tile([PART, CT, C], BT)
            wots = wp.tile([PART, CT, C], BT)

            hts = [hp.tile([PART, N], BT, tag=f"h{ci}", name=f"h{ci}") for ci in range(CT)]

            # ---- Phase A+B: GroupNorm stats + normalize into h ----
            with (
                tc.tile_pool(name="stats", bufs=2) as sp,
                tc.tile_pool(name="coef", bufs=1) as cfp,
                tc.tile_pool(name="xst", bufs=4) as xp,
                tc.tile_pool(name="pst", bufs=2, space="PSUM") as pp,
            ):
                acoef = cfp.tile([PART, CT], F32)
                bcoef = cfp.tile([PART, CT], F32)

                def stats_halves(xt, s2):
                    # DVE bn_stats on the first half, ACT sum/sumsq
                    # (accum_out) on the second half, combined into
                    # (mean, E[x^2]) per channel
                    xr = xt[:, 0:N // 2].rearrange("p (s f) -> p s f", f=512)
                    st6 = sp.tile([PART, 4, 6], F32, tag="st6")
                    for s in range(4):
                        nc.vector.bn_stats(out=st6[:, s, :], in_=xr[:, s, :])
                    mv = sp.tile([PART, 2], F32, tag="mv")
                    nc.vector.bn_aggr(out=mv, in_=st6)
                    ssc = sp.tile([PART, N // 2], BT, tag="ssc")
                    asum = sp.tile([PART, 1], F32, tag="asum")
                    asq = sp.tile([PART, 1], F32, tag="asq")
                    nc.scalar.activation(out=ssc, in_=xt[:, N // 2:],
                                         func=AF.Identity, accum_out=asum)
                    ssc2 = sp.tile([PART, N // 2], BT, tag="ssc")
                    nc.scalar.activation(out=ssc2, in_=xt[:, N // 2:],
                                         func=AF.Square, accum_out=asq)
                    # mean = mean_a/2 + sum_b/N
                    tmA = sp.tile([PART, 1], F32, tag="tmA")
                    nc.vector.tensor_scalar(
                        out=tmA, in0=asum, scalar1=1.0 / N, scalar2=None,
                        op0=OP.mult)
                    nc.vector.tensor_scalar(
                        out=s2[:, 0:1], in0=mv[:, 0:1], scalar1=0.5,
                        scalar2=None, op0=OP.mult)
                    nc.vector.tensor_add(out=s2[:, 0:1], in0=s2[:, 0:1], in1=tmA)
                    # E[x^2] = (var_a + mean_a^2)/2 + sumsq_b/N
                    tmB = sp.tile([PART, 1], F32, tag="tmB")
                    nc.vector.tensor_tensor(
                        out=tmB, in0=mv[:, 0:1], in1=mv[:, 0:1], op=OP.mult)
                    nc.vector.tensor_add(out=tmB, in0=tmB, in1=mv[:, 1:2])
                    tmC = sp.tile([PART, 1], F32, tag="tmC")
                    nc.vector.tensor_scalar(
                        out=tmC, in0=asq, scalar1=1.0 / N, scalar2=None,
                        op0=OP.mult)
                    nc.vector.tensor_scalar(
                        out=tmB, in0=tmB, scalar1=0.5, scalar2=None,
                        op0=OP.mult)
                    nc.vector.tensor_add(out=s2[:, 1:2], in0=tmB, in1=tmC)

                def stats_bn(xt, s2):
                    xr8 = xt.rearrange("p (s f) -> p s f", f=512)
                    st8 = sp.tile([PART, 8, 6], F32, tag="st6")
                    for s in range(8):
                        nc.vector.bn_stats(out=st8[:, s, :], in_=xr8[:, s, :])
                    mv8 = sp.tile([PART, 2], F32, tag="mv")
                    nc.vector.bn_aggr(out=mv8, in_=st8)
                    nc.vector.tensor_copy(out=s2[:, 0:1], in_=mv8[:, 0:1])
                    nc.vector.tensor_tensor(
                        out=s2[:, 1:2], in0=mv8[:, 0:1], in1=mv8[:, 0:1],
                        op=OP.mult)
                    nc.vector.tensor_add(out=s2[:, 1:2], in0=s2[:, 1:2],
                                         in1=mv8[:, 1:2])

                def stats_act(xt, s2):
                    ssc = sp.tile([PART, N], BT, tag="ssc")
                    asum = sp.tile([PART, 1], F32, tag="asum")
                    asq = sp.tile([PART, 1], F32, tag="asq")
                    nc.scalar.activation(out=ssc, in_=xt, func=AF.Identity,
                                         accum_out=asum)
                    ssc2 = sp.tile([PART, N], BT, tag="ssc")
                    nc.scalar.activation(out=ssc2, in_=xt, func=AF.Square,
                                         accum_out=asq)
                    nc.vector.tensor_scalar(
                        out=s2[:, 0:1], in0=asum, scalar1=1.0 / N,
                        scalar2=None, op0=OP.mult)
                    nc.vector.tensor_scalar(
                        out=s2[:, 1:2], in0=asq, scalar1=1.0 / N,
                        scalar2=None, op0=OP.mult)

                for ci in range(CT):
                    xt = xp.tile([PART, N], BT, tag="x")
                    nc.sync.dma_start(out=xt, in_=xb[ci * PART:(ci + 1) * PART, :])
                    s2 = sp.tile([PART, 2], F32R, tag="s2")
                    if stats_mode == 'halves':
                        stats_halves(xt, s2)
                    elif stats_mode == 'tile3act' and ci == CT - 1:
                        stats_act(xt, s2)
                    else:
                        stats_bn(xt, s2)
                    # group means over 16-channel blocks: [8, 2]
                    gp_ = pp.tile([8, 2], F32, tag="gp")
                    nc.tensor.matmul(gp_, lhsT=gm, rhs=s2,
                                     start=True, stop=True)
                    gs = sp.tile([8, 2], F32R, tag="gs")
                    nc.vector.tensor_copy(out=gs, in_=gp_)
                    msq = sp.tile([8, 1], F32, tag="msq")
                    nc.vector.tensor_tensor(
                        out=msq, in0=gs[:, 0:1], in1=gs[:, 0:1], op=OP.mult)
                    nc.vector.tensor_sub(out=gs[:, 1:2], in0=gs[:, 1:2], in1=msq)
                    nc.scalar.activation(out=gs[:, 1:2], in_=gs[:, 1:2],
                                         func=AF.Sqrt, bias=epst[0:8])
                    with nc.allow_low_precision(
                            reason="fp32r rounding for PE broadcast matmul"):
                        nc.vector.reciprocal(out=gs[:, 1:2], in_=gs[:, 1:2])
                    # broadcast per-group (mean, rstd) back to channels
                    cb = pp.tile([PART, 2], F32, tag="cb")
                    nc.tensor.matmul(cb, lhsT=gmt, rhs=gs,
                                     start=True, stop=True)
                    nc.vector.tensor_tensor(
                        out=acoef[:, ci:ci + 1], in0=cb[:, 1:2],
                        in1=gwt[:, ci:ci + 1], op=OP.mult)
                    tmpb = sp.tile([PART, 1], F32, tag="tmpb")
                    nc.vector.tensor_tensor(
                        out=tmpb, in0=cb[:, 0:1], in1=acoef[:, ci:ci + 1], op=OP.mult)
                    nc.vector.tensor_sub(
                        out=bcoef[:, ci:ci + 1], in0=gbt[:, ci:ci + 1], in1=tmpb)
                    # h = x * a + b
                    nc.vector.tensor_scalar(
                        out=hts[ci], in0=xt,
                        scalar1=acoef[:, ci:ci + 1], scalar2=bcoef[:, ci:ci + 1],
                        op0=OP.mult, op1=OP.add)

            # big weight loads deferred past the critical xb tiles
            for j in range(CT):
                nc.sync.dma_start(out=wvts[:, j, :], in_=wvt[j * PART:(j + 1) * PART, :])
            for j in range(CT):
                nc.sync.dma_start(out=mts[:, j, :], in_=mt[j * PART:(j + 1) * PART, :])
            for j in range(CT):
                nc.sync.dma_start(out=wots[:, j, :], in_=wot[j * PART:(j + 1) * PART, :])
            nc.sync.dma_start(out=ones, in_=onesd)
            nc.sync.dma_start(out=onesf, in_=onesfd)

            # ---- Phase C: v^T tiles ----
            with tc.tile_pool(name="vtp", bufs=1) as vp:
                vts = vp.tile([PART, NKT, CH], BT)
                with tc.tile_pool(name="vps", bufs=4, space="PSUM") as vpp:
                    for t in range(NKT):
                        vps = vpp.tile([PART, CH], F32, tag="vps")
                        for ci in range(CT):
                            nc.tensor.matmul(
                                vps,
                                lhsT=hts[ci][:, t * PART:(t + 1) * PART],
                                rhs=wvts[:, ci, :],
                                start=(ci == 0), stop=(ci == CT - 1))
                        nc.vector.tensor_copy(out=vts[:, t, :], in_=vps)

                # ---- Phase D+E: attention + out-projection, per nq-chunk ----
                with (
                    tc.tile_pool(name="ujp", bufs=2) as up,
                    tc.tile_pool(name="ep", bufs=6) as ep,
                    tc.tile_pool(name="attp", bufs=1) as ap_,
                    tc.tile_pool(name="rcp", bufs=2) as rp,
                    tc.tile_pool(name="xrp", bufs=2) as xrp,
                    tc.tile_pool(name="otp", bufs=2) as otp,
                    tc.tile_pool(name="oup", bufs=1, space="PSUM") as oup,
                    tc.tile_pool(name="stp", bufs=2, space="PSUM") as stp,
                    tc.tile_pool(name="ddp", bufs=1, space="PSUM") as ddp,
                    tc.tile_pool(name="ddap", bufs=2) as ddap,
                    tc.tile_pool(name="upp", bufs=1, space="PSUM") as upp,
                ):
                    def compute_u(jc):
                        # u_jc = M h[:, chunk jc] + c0
                        ut = up.tile([PART, CT, CH], BT, tag="uj", name=f"uj{jc}")
                        sl = slice(jc * CH, (jc + 1) * CH)
                        for i in range(CT):
                            ups = upp.tile([PART, CH], F32, tag="up")
                            for jj in range(CT):
                                nc.tensor.matmul(
                                    ups,
                                    lhsT=mts[:, jj, i * PART:(i + 1) * PART],
                                    rhs=hts[jj][:, sl],
                                    start=(jj == 0), stop=(jj == CT - 1))
                            nc.vector.tensor_scalar(
                                out=ut[:, i, :], in0=ups,
                                scalar1=c0s[:, i:i + 1], scalar2=None, op0=OP.add)
                        return ut

                    uj_next = compute_u(0)
                    for j in range(JCH):
                        jsl = slice(j * CH, (j + 1) * CH)
                        uj = uj_next
                        # attention accumulation over key tiles
                        ou = [oup.tile([PART, CH], F32, tag=f"ou{m}", name=f"ou{m}_{j}") for m in range(CT)]
                        dda = ddap.tile([PART, CH], F32R, tag="dda")
                        for t in range(NKT):
                            st = stp.tile([PART, CH], F32, tag="st")
                            for ci in range(CT):
                                nc.tensor.matmul(
                                    st,
                                    lhsT=hts[ci][:, t * PART:(t + 1) * PART],
                                    rhs=uj[:, ci, :],
                                    start=(ci == 0), stop=(ci == CT - 1))
                            et = ep.tile([PART, CH], BT, tag="et")
                            nc.scalar.activation(out=et, in_=st, func=AF.Exp, scale=SCALE)
                            etr = et[:]
                            for m in range(CT):
                                nc.tensor.matmul(
                                    ou[m],
                                    lhsT=vts[:, t, m * PART:(m + 1) * PART],
                                    rhs=etr,
                                    start=(t == 0), stop=(t == NKT - 1))
                            if t == 0:
                                nc.vector.tensor_copy(out=dda, in_=etr)
                            else:
                                nc.vector.tensor_add(out=dda, in0=dda, in1=etr)
                            if t == NKT - 5 and j + 1 < JCH:
                                uj_next = compute_u(j + 1)
                        # copy unnormalized numerators to SBUF; the per-column
                        # 1/D commutes with the out-projection, so the
                        # reciprocal runs off the PE critical path
                        att = ap_.tile([PART, CT, CH], BT, tag="att")
                        for m in range(CT):
                            if m % 2 == 0:
                                nc.vector.tensor_copy(out=att[:, m, :], in_=ou[m])
                            else:
                                nc.scalar.copy(out=att[:, m, :], in_=ou[m])
                        ddr = ddp.tile([PART, CH], F32, tag="ddr")
                        nc.tensor.matmul(ddr, lhsT=onesf, rhs=dda,
                                         start=True, stop=True)
                        rc = rp.tile([PART, CH], F32, tag="rc")
                        nc.vector.reciprocal(out=rc, in_=ddr)
                        # out-projection + normalize + bias + residual
                        for m in range(CT):
                            fp = oup.tile([PART, CH], F32, tag=f"ou{m}")
                            for ci in range(CT):
                                nc.tensor.matmul(
                                    fp,
                                    lhsT=wots[:, ci, m * PART:(m + 1) * PART],
                                    rhs=att[:, ci, :],
                                    start=(ci == 0), stop=(ci == CT - 1))
                            xr_ = xrp.tile([PART, CH], F32, tag="xr")
                            nc.sync.dma_start(
                                out=xr_, in_=xa[m * PART:(m + 1) * PART, jsl])
                            ot = otp.tile([PART, CH], F32, tag="ot")
                            nc.vector.tensor_mul(out=ot, in0=fp, in1=rc)
                            nc.vector.tensor_scalar(
                                out=ot, in0=ot,
                                scalar1=bo2s[:, m:m + 1], scalar2=None, op0=OP.add)
                            nc.vector.tensor_add(out=ot, in0=ot, in1=xr_)
                            nc.sync.dma_start(
                                out=y[m * PART:(m + 1) * PART, jsl], in_=ot)
    if split_waits:
        split_multi_waits(nc)
    return nc


def prep_inputs(x, gn_w, gn_b, qkv_w, qkv_b, out_w, out_b):
    x = np.asarray(x, np.float32)
    gn_w = np.asarray(gn_w, np.float32)
    gn_b = np.asarray(gn_b, np.float32)
    qkv_w = np.asarray(qkv_w, np.float32)
    qkv_b = np.asarray(qkv_b, np.float32)
    out_w = np.asarray(out_w, np.float32)
    out_b = np.asarray(out_b, np.float32)

    Wq, Wk, Wv = qkv_w[0:C], qkv_w[C:2 * C], qkv_w[2 * C:3 * C]
    bq, bv = qkv_b[0:C], qkv_b[2 * C:3 * C]
    bf16 = ml_dtypes.bfloat16
    mt = np.ascontiguousarray((Wq.T @ Wk).astype(bf16))          # (M = Wk^T Wq).T
    wvt = np.ascontiguousarray(Wv.T.astype(bf16))
    wot = np.ascontiguousarray(out_w.T.astype(bf16))
    c0 = (Wk.T @ bq).astype(np.float32)
    bo2 = (out_w @ bv + out_b).astype(np.float32)

    def coltiles(v):
        return np.ascontiguousarray(v.reshape(CT, PART).T, dtype=np.float32)

    gmat = np.zeros((PART, 8), np.float32)
    gmatt = np.zeros((8, PART), np.float32)
    for p in range(PART):
        gmat[p, p // 16] = 1.0 / 16.0
        gmatt[p // 16, p] = 1.0
    shared = {
        "mt": mt, "wvt": wvt, "wot": wot,
        "gw": coltiles(gn_w), "gb": coltiles(gn_b),
        "c0t": coltiles(c0), "bo2t": coltiles(bo2),
        "gmat": gmat, "gmatt": gmatt,
        "onesd": np.ones((PART, PART), ml_dtypes.bfloat16),
        "onesfd": np.ones((PART, PART), np.float32),
    }
    in_maps = []
    for core in range(8):
        br, hf = divmod(core, 2)
        xa = x[br].reshape(C, N)
        if hf:
            xa = np.concatenate([xa[:, NQ:], xa[:, :NQ]], axis=1)
        xa = np.ascontiguousarray(xa, dtype=np.float32)
        in_maps.append({"xa": xa, "xb": xa.astype(ml_dtypes.bfloat16), **shared})
    return in_maps


def assemble_output(results, b=4, hh=64, ww=64):
    out = np.zeros((b, C, N), np.float32)
    for core in range(8):
        br, hf = divmod(core, 2)
        out[br][:, hf * NQ:(hf + 1) * NQ] = results[core]["y"]
    return out.reshape(b, C, hh, ww)


def kernel(x, gn_w, gn_b, qkv_w, qkv_b, out_w, out_b):
    from concourse import bass_utils
    in_maps = prep_inputs(x, gn_w, gn_b, qkv_w, qkv_b, out_w, out_b)
    nc = build_program()
    res = bass_utils.run_bass_kernel_spmd(nc, in_maps, core_ids=list(range(8)))
    return assemble_output(res.results)



# revision 4
# speedup vs baseline: 1.4262x; 1.4262x over previous
"""Trainium2 Bass kernel for nn_AttentionBlock (GroupNorm + single-head
self-attention over 64x64 spatial + out-projection + residual).

Sharding: 8 cores = 4 batches x 2 query-halves. Each core receives its
batch's x as [512, 4096] (channels x pixels), rotated so that its own
2048 query pixels are columns 0:2048. GroupNorm stats / keys / values
span all 4096 pixels (invariant to the rotation), so the program is
identical on every core (pure SPMD, no collectives); the host gathers
the 8 [512, 2048] outputs back into (4, 512, 64, 64).

Algebraic restructuring (host-precomputed):
  - scores^T = h^T (A h + c0),  A = Wk^T Wq, c0 = Wk^T bq: k/q are never
    materialized (the k-bias term is constant within each softmax
    column and cancels exactly).
  - v^T = (Wv h)^T with no bias; bv commutes through the attention
    average and folds into bo2 = out_w @ bv + out_b.
  - softmax without max-subtraction; exp is biased by -ESHIFT so that
    E stays within fp8-e4m3 range (the shift cancels exactly in the
    normalization since the denominator is built from the same E).

Precision: the four large matmul families (u = A h, v = Wv h,
scores = h^T u, numerator = v E) run in fp8-e4m3 with
perf_mode=DoubleRow (two contraction elements per PE cell -> K=256 per
matmul, 2x PE throughput). A and Wv are pre-scaled by 16 to sit in
e4m3's normal range; the scales are folded into the exp scale and the
softmax reciprocal (the denominator's ones-matmul uses stationary
value 16 to absorb the v-scale for free). The out-projection stays
bf16 on the softmax-normalized attention output. GroupNorm statistics
are fp32 on a bf16 copy of x; the residual uses the original fp32 x.
Measured relative error vs the fp32 reference: ~3e-3.

The softmax denominator is accumulated on the PE (one DoubleRow
ones-matmul per key-tile-pair into a dedicated PSUM bank) instead of
DVE adds, keeping the vector engine far below the PE roofline. The
weighted-value matmuls lag the score matmuls by one key-tile-pair so
the Exp activation latency is hidden; per-chunk u-projections and the
previous chunk's out-projection are injected into the key loop so
chunk boundaries stay dense on the PE.

Infrastructure workarounds (this container's walrus accepts at most
one sync-wait per instruction): Tile's kernel-tail drain waits are
re-emitted as single-wait NOPs, and a post-scheduling pass hoists
extra waits from any instruction onto preceding single-wait NOPs.
"""

import numpy as np
import ml_dtypes

import concourse.bass as bass
import concourse.bass_isa as bass_isa
from concourse import library_config
import concourse.mybir as mybir
import concourse.tile as tile
from concourse.tile_scheduler import N_PROCS
from concourse.vector_clock import ScopedClock, VectorClock

F32 = mybir.dt.float32
F32R = mybir.dt.float32r
BT = mybir.dt.bfloat16
FP8 = mybir.dt.float8e4
AF = mybir.ActivationFunctionType
OP = mybir.AluOpType
DR = mybir.MatmulPerfMode.DoubleRow

PART = 128
C = 512          # channels
N = 4096         # pixels per batch
NQ = 2048        # query pixels per core
CT = C // PART   # 4 channel tiles
NKT = N // PART  # 32 key tiles
NTP = NKT // 2   # 16 key tile pairs
CH = 512         # nq chunk width
JCH = NQ // CH   # 4 chunks
EPS = 1e-5
SCALE = float(C) ** -0.5
WSCALE = 16.0    # fp8 pre-scale on A and Wv
ESHIFT = 2.0     # exp bias: E = exp(s*SCALE - ESHIFT), cancels in softmax


def _patched_drain_and_barrier(self, tick_clock, wait_clock):
    # Walrus in this container accepts at most one sync-wait per
    # instruction; Tile's stock exit path stacks every outstanding
    # proc's wait on a single SP Drain. Emit one single-wait NOP per
    # proc instead, then a wait-free drain.
    nc = self.nc
    gc = tick_clock.global_clock
    for p in range(N_PROCS):
        t = gc[p]
        if t <= 0:
            continue
        vc = VectorClock([t if q == p else 0 for q in range(N_PROCS)])
        nop = nc.sync.nop(nofuse=True, hint=f"drainwait{p}")
        wait_clock.add_sem_waits(nop.ins, ScopedClock({None: vc}))
    nc.sync.drain()

    nc.all_engine_barrier()
    assert self.sems is not None
    popped = nc._tile_sem_poison_stack.pop()
    assert popped is self._sem_poison
    nc.clear_and_free_semaphores(list(self.sems.allocated().values()))


def apply_tile_patch():
    tile.TileContext._drain_and_barrier = _patched_drain_and_barrier


def split_multi_waits(nc):
    """Walrus in this container accepts at most one sync-wait command per
    instruction. Tile's wait-assignment freely stacks several. Hoist all
    but the last wait of each instruction onto single-wait NOPs inserted
    immediately before it on the same engine (engine blocks on each in
    turn, so the gating is equivalent)."""
    k = 0
    for fn in nc.m.functions:
        for bb in fn.blocks:
            il = bb.instructions
            i = 0
            while i < len(il):
                inst = il[i]
                si = inst.sync_info
                waits = list(si.on_wait) if si and si.on_wait else []
                if len(waits) > 1:
                    for w in waits[:-1]:
                        nop = mybir.InstNoOp(name=f"I-waitsplit-{k}")
                        k += 1
                        nop.engine = inst.engine
                        nop.sync_info = mybir.SyncInfo(on_wait=[w], on_update=[])
                        il.insert(i, nop)
                        i += 1
                    si.on_wait = [waits[-1]]
                    inst.sync_info = si
                i += 1


def build_program(split_waits=True):
    apply_tile_patch()
    nc = bass.Bass(name="attnblk")
    xa = nc.dram_tensor("xa", [C, N], F32, kind="ExternalInput").ap()
    xb = nc.dram_tensor("xb", [C, N], BT, kind="ExternalInput").ap()
    mt8d = nc.dram_tensor("mt8", [C, C], FP8, kind="ExternalInput").ap()
    wv8d = nc.dram_tensor("wv8", [C, C], FP8, kind="ExternalInput").ap()
    wotd = nc.dram_tensor("wot", [C, C], BT, kind="ExternalInput").ap()
    gw = nc.dram_tensor("gw", [PART, CT], F32, kind="ExternalInput").ap()
    gb = nc.dram_tensor("gb", [PART, CT], F32, kind="ExternalInput").ap()
    c0t = nc.dram_tensor("c0t", [PART, CT], F32, kind="ExternalInput").ap()
    bo2t = nc.dram_tensor("bo2t", [PART, CT], F32, kind="ExternalInput").ap()
    gmat = nc.dram_tensor("gmat", [PART, 8], F32R, kind="ExternalInput").ap()
    gmatt = nc.dram_tensor("gmatt", [8, PART], F32R, kind="ExternalInput").ap()
    onesd = nc.dram_tensor("onesd", [PART, 2 * PART], FP8, kind="ExternalInput").ap()
    y = nc.dram_tensor("y", [C, NQ], F32, kind="ExternalOutput").ap()

    with tile.TileContext(nc) as tc:
        with (
            tc.tile_pool(name="const", bufs=1) as cp,
            tc.tile_pool(name="wts", bufs=1) as wp,
            tc.tile_pool(name="hp", bufs=1) as hp,
            tc.tile_pool(name="vtp", bufs=1) as vp,
        ):
            gwt = cp.tile([PART, CT], F32)
            nc.sync.dma_start(out=gwt, in_=gw)
            gbt = cp.tile([PART, CT], F32)
            nc.sync.dma_start(out=gbt, in_=gb)
            c0s = cp.tile([PART, CT], F32)
            nc.sync.dma_start(out=c0s, in_=c0t)
            bo2s = cp.tile([PART, CT], F32)
            nc.sync.dma_start(out=bo2s, in_=bo2t)
            gm = cp.tile([PART, 8], F32R)
            nc.sync.dma_start(out=gm, in_=gmat)
            gmt = cp.tile([8, PART], F32R)
            nc.sync.dma_start(out=gmt, in_=gmatt)
            ones8 = cp.tile([PART, 2, PART], FP8)
            nc.sync.dma_start(out=ones8, in_=onesd)
            epst = cp.tile([PART, 1], F32)
            nc.vector.memset(epst, EPS)
            esh = cp.tile([PART, 1], F32)
            nc.vector.memset(esh, -ESHIFT)

            # fp8 weights are tiny (256KB each): load before x
            mts8 = wp.tile([PART, CT, C], FP8)
            for j in range(CT):
                nc.sync.dma_start(out=mts8[:, j, :], in_=mt8d[j * PART:(j + 1) * PART, :])
            wvs8 = wp.tile([PART, CT, C], FP8)
            for j in range(CT):
                nc.sync.dma_start(out=wvs8[:, j, :], in_=wv8d[j * PART:(j + 1) * PART, :])
            wots = wp.tile([PART, CT, C], BT)

            h8 = hp.tile([PART, CT, N], FP8, name="h8")
            v8 = vp.tile([PART, NTP, 2, CH], FP8, name="v8")

            # ---- Phase A: GroupNorm stats + normalize into fp8 h ----
            with (
                tc.tile_pool(name="stats", bufs=2) as sp,
                tc.tile_pool(name="coef", bufs=1) as cfp,
                tc.tile_pool(name="xst", bufs=4) as xp,
                tc.tile_pool(name="pst", bufs=2, space="PSUM") as pp,
            ):
                acoef = cfp.tile([PART, CT], F32)
                bcoef = cfp.tile([PART, CT], F32)

                def stats_halves(xt, s2):
                    # DVE bn_stats on the first half, ACT sum/sumsq
                    # (accum_out) on the second half, combined into
                    # (mean, E[x^2]) per channel
                    xr = xt[:, 0:N // 2].rearrange("p (s f) -> p s f", f=512)
                    st6 = sp.tile([PART, 4, 6], F32, tag="st6")
                    for s in range(4):
                        nc.vector.bn_stats(out=st6[:, s, :], in_=xr[:, s, :])
                    mv = sp.tile([PART, 2], F32, tag="mv")
                    nc.vector.bn_aggr(out=mv, in_=st6)
                    ssc = sp.tile([PART, N // 2], BT, tag="ssc")
                    asum = sp.tile([PART, 1], F32, tag="asum")
                    asq = sp.tile([PART, 1], F32, tag="asq")
                    nc.scalar.activation(out=ssc, in_=xt[:, N // 2:],
                                         func=AF.Identity, accum_out=asum)
                    ssc2 = sp.tile([PART, N // 2], BT, tag="ssc")
                    nc.scalar.activation(out=ssc2, in_=xt[:, N // 2:],
                                         func=AF.Square, accum_out=asq)
                    # mean = mean_a/2 + sum_b/N
                    tmA = sp.tile([PART, 1], F32, tag="tmA")
                    nc.vector.tensor_scalar(
                        out=tmA, in0=asum, scalar1=1.0 / N, scalar2=None,
                        op0=OP.mult)
                    nc.vector.tensor_scalar(
                        out=s2[:, 0:1], in0=mv[:, 0:1], scalar1=0.5,
                        scalar2=None, op0=OP.mult)
                    nc.vector.tensor_add(out=s2[:, 0:1], in0=s2[:, 0:1], in1=tmA)
                    # E[x^2] = (var_a + mean_a^2)/2 + sumsq_b/N
                    tmB = sp.tile([PART, 1], F32, tag="tmB")
                    nc.vector.tensor_tensor(
                        out=tmB, in0=mv[:, 0:1], in1=mv[:, 0:1], op=OP.mult)
                    nc.vector.tensor_add(out=tmB, in0=tmB, in1=mv[:, 1:2])
                    tmC = sp.tile([PART, 1], F32, tag="tmC")
                    nc.vector.tensor_scalar(
                        out=tmC, in0=asq, scalar1=1.0 / N, scalar2=None,
                        op0=OP.mult)
                    nc.vector.tensor_scalar(
                        out=tmB, in0=tmB, scalar1=0.5, scalar2=None,
                        op0=OP.mult)
                    nc.vector.tensor_add(out=s2[:, 1:2], in0=tmB, in1=tmC)

                for ci in range(CT):
                    xt = xp.tile([PART, N], BT, tag="x")
                    nc.sync.dma_start(
                        out=xt[:, 0:N // 2],
                        in_=xb[ci * PART:(ci + 1) * PART, 0:N // 2])
                    nc.sync.dma_start(
                        out=xt[:, N // 2:],
                        in_=xb[ci * PART:(ci + 1) * PART, N // 2:])
                    s2 = sp.tile([PART, 2], F32R, tag="s2")
                    stats_halves(xt, s2)
                    # group means over 16-channel blocks: [8, 2]
                    gp_ = pp.tile([8, 2], F32, tag="gp")
                    nc.tensor.matmul(gp_, lhsT=gm, rhs=s2,
                                     start=True, stop=True)
                    gs = sp.tile([8, 2], F32R, tag="gs")
                    nc.vector.tensor_copy(out=gs, in_=gp_)
                    msq = sp.tile([8, 1], F32, tag="msq")
                    nc.vector.tensor_tensor(
                        out=msq, in0=gs[:, 0:1], in1=gs[:, 0:1], op=OP.mult)
                    nc.vector.tensor_sub(out=gs[:, 1:2], in0=gs[:, 1:2], in1=msq)
                    nc.scalar.activation(out=gs[:, 1:2], in_=gs[:, 1:2],
                                         func=AF.Sqrt, bias=epst[0:8])
                    with nc.allow_low_precision(
                            reason="fp32r rounding for PE broadcast matmul"):
                        nc.vector.reciprocal(out=gs[:, 1:2], in_=gs[:, 1:2])
                    # broadcast per-group (mean, rstd) back to channels
                    cb = pp.tile([PART, 2], F32, tag="cb")
                    nc.tensor.matmul(cb, lhsT=gmt, rhs=gs,
                                     start=True, stop=True)
                    nc.vector.tensor_tensor(
                        out=acoef[:, ci:ci + 1], in0=cb[:, 1:2],
                        in1=gwt[:, ci:ci + 1], op=OP.mult)
                    tmpb = sp.tile([PART, 1], F32, tag="tmpb")
                    nc.vector.tensor_tensor(
                        out=tmpb, in0=cb[:, 0:1], in1=acoef[:, ci:ci + 1], op=OP.mult)
                    nc.vector.tensor_sub(
                        out=bcoef[:, ci:ci + 1], in0=gbt[:, ci:ci + 1], in1=tmpb)
                    # h = x * a + b, straight to fp8
                    nc.vector.tensor_scalar(
                        out=h8[:, ci, :], in0=xt,
                        scalar1=acoef[:, ci:ci + 1], scalar2=bcoef[:, ci:ci + 1],
                        op0=OP.mult, op1=OP.add)

            # out-projection weights deferred past the critical xb tiles
            for j in range(CT):
                nc.sync.dma_start(out=wots[:, j, :], in_=wotd[j * PART:(j + 1) * PART, :])

            # ---- Phase C: v^T tiles (fp8 DoubleRow) ----
            with tc.tile_pool(name="vps", bufs=4, space="PSUM") as vpp:
                for t in range(NKT):
                    vps = vpp.tile([PART, CH], F32, tag="vps")
                    ksl = slice(t * PART, (t + 1) * PART)
                    for a in range(2):
                        nc.tensor.matmul(
                            vps,
                            lhsT=h8[:, 2 * a:2 * a + 2, ksl],
                            rhs=wvs8[:, 2 * a:2 * a + 2, :],
                            start=(a == 0), stop=(a == 1), perf_mode=DR)
                    nc.vector.tensor_copy(out=v8[:, t // 2, t % 2, :], in_=vps)

            # ---- Phase D+E: attention + out-projection, per nq-chunk ----
            with (
                tc.tile_pool(name="ujp", bufs=2) as up,
                tc.tile_pool(name="ep", bufs=3) as ep,
                tc.tile_pool(name="attp", bufs=2) as ap_,
                tc.tile_pool(name="rcp", bufs=2) as rp,
                tc.tile_pool(name="xrp", bufs=8) as xrp,
                tc.tile_pool(name="otp", bufs=2) as otp,
                tc.tile_pool(name="oup", bufs=1, space="PSUM") as oup,
                tc.tile_pool(name="stp", bufs=2, space="PSUM") as stp,
                tc.tile_pool(name="ddp", bufs=1, space="PSUM") as ddp,
                tc.tile_pool(name="fpp", bufs=1, space="PSUM") as fpp,
            ):
                u8s = {}
                ous = {}
                dds = {}
                atts = {}
                xrbs = {}

                def emit_u(jc):
                    # u_jc = A16 h[:, chunk jc] + 16*c0, fp8
                    ut = up.tile([PART, CT, CH], FP8, tag="uj", name=f"uj{jc}")
                    sl = slice(jc * CH, (jc + 1) * CH)
                    for i in range(CT):
                        ups = fpp.tile([PART, CH], F32, tag="fpu")
                        for a in range(2):
                            nc.tensor.matmul(
                                ups,
                                lhsT=mts8[:, 2 * a:2 * a + 2, i * PART:(i + 1) * PART],
                                rhs=h8[:, 2 * a:2 * a + 2, sl],
                                start=(a == 0), stop=(a == 1), perf_mode=DR)
                        nc.vector.tensor_scalar(
                            out=ut[:, i, :], in0=ups,
                            scalar1=c0s[:, i:i + 1], scalar2=None, op0=OP.add)
                    u8s[jc] = ut

                def emit_ou(j, et, tp):
                    if tp == 0:
                        ous[j] = [oup.tile([PART, CH], F32, tag=f"ou{m}",
                                           name=f"ou{m}_{j}") for m in range(CT)]
                        dds[j] = ddp.tile([PART, CH], F32, tag="dd", name=f"dd{j}")
                    for m in range(CT):
                        nc.tensor.matmul(
                            ous[j][m],
                            lhsT=v8[:, tp, :, m * PART:(m + 1) * PART],
                            rhs=et,
                            start=(tp == 0), stop=(tp == NTP - 1), perf_mode=DR)
                    nc.tensor.matmul(
                        dds[j], lhsT=ones8, rhs=et,
                        start=(tp == 0), stop=(tp == NTP - 1), perf_mode=DR)

                def emit_xrb(j, m):
                    jsl = slice(j * CH, (j + 1) * CH)
                    xr_ = xrp.tile([PART, CH], F32, tag="xr")
                    nc.sync.dma_start(out=xr_, in_=xa[m * PART:(m + 1) * PART, jsl])
                    xrb = xrp.tile([PART, CH], F32, tag="xrb")
                    nc.vector.tensor_scalar(
                        out=xrb, in0=xr_, scalar1=bo2s[:, m:m + 1],
                        scalar2=None, op0=OP.add)
                    xrbs[(j, m)] = xrb

                def emit_rc_att(j):
                    rc = rp.tile([PART, CH], F32, tag="rc")
                    nc.vector.reciprocal(out=rc, in_=dds[j])
                    att = ap_.tile([PART, CT, CH], BT, tag="att", name=f"att{j}")
                    for m in range(CT):
                        nc.vector.tensor_tensor(
                            out=att[:, m, :], in0=ous[j][m], in1=rc, op=OP.mult)
                    atts[j] = att

                def emit_fp(j, m, pool, tag):
                    # out-projection tile m of chunk j (bf16) + bias/residual + store
                    jsl = slice(j * CH, (j + 1) * CH)
                    fp = pool.tile([PART, CH], F32, tag=tag)
                    for ci in range(CT):
                        nc.tensor.matmul(
                            fp,
                            lhsT=wots[:, ci, m * PART:(m + 1) * PART],
                            rhs=atts[j][:, ci, :],
                            start=(ci == 0), stop=(ci == CT - 1))
                    ot = otp.tile([PART, CH], F32, tag="ot")
                    nc.vector.tensor_add(out=ot, in0=fp, in1=xrbs[(j, m)])
                    nc.sync.dma_start(out=y[m * PART:(m + 1) * PART, jsl], in_=ot)

                emit_u(0)
                for j in range(JCH):
                    uj = u8s[j]
                    prev_et = None
                    for tp in range(NTP):
                        st_ = []
                        for half in range(2):
                            t = 2 * tp + half
                            ksl = slice(t * PART, (t + 1) * PART)
                            st = stp.tile([PART, CH], F32, tag="st")
                            for a in range(2):
                                nc.tensor.matmul(
                                    st,
                                    lhsT=h8[:, 2 * a:2 * a + 2, ksl],
                                    rhs=uj[:, 2 * a:2 * a + 2, :],
                                    start=(a == 0), stop=(a == 1), perf_mode=DR)
                            st_.append(st)
                        if prev_et is not None:
                            emit_ou(j, prev_et, tp - 1)
                        if j > 0 and tp < CT:
                            emit_fp(j - 1, tp, fpp, "fpu")
                        if CT <= tp < 2 * CT:
                            emit_xrb(j, tp - CT)
                        et = ep.tile([PART, 2, CH], FP8, tag="et")
                        for half in range(2):
                            nc.scalar.activation(
                                out=et[:, half, :], in_=st_[half],
                                func=AF.Exp, scale=SCALE / WSCALE, bias=esh)
                        prev_et = et
                        if tp == NTP - 3 and j + 1 < JCH:
                            emit_u(j + 1)
                    emit_ou(j, prev_et, NTP - 1)
                    emit_rc_att(j)
                # final chunk's out-projection (st banks are free by now)
                for m in range(CT):
                    emit_fp(JCH - 1, m, stp, "st")
    if split_waits:
        split_multi_waits(nc)
    return nc


def prep_inputs(x, gn_w, gn_b, qkv_w, qkv_b, out_w, out_b):
    x = np.asarray(x, np.float32)
    gn_w = np.asarray(gn_w, np.float32)
    gn_b = np.asarray(gn_b, np.float32)
    qkv_w = np.asarray(qkv_w, np.float32)
    qkv_b = np.asarray(qkv_b, np.float32)
    out_w = np.asarray(out_w, np.float32)
    out_b = np.asarray(out_b, np.float32)

    Wq, Wk, Wv = qkv_w[0:C], qkv_w[C:2 * C], qkv_w[2 * C:3 * C]
    bq, bv = qkv_b[0:C], qkv_b[2 * C:3 * C]
    bf16 = ml_dtypes.bfloat16
    e4 = ml_dtypes.float8_e4m3
    mt8 = np.ascontiguousarray((WSCALE * (Wq.T @ Wk)).astype(e4))
    wv8 = np.ascontiguousarray((WSCALE * Wv.T).astype(e4))
    wot = np.ascontiguousarray(out_w.T.astype(bf16))
    c0 = (WSCALE * (Wk.T @ bq)).astype(np.float32)
    bo2 = (out_w @ bv + out_b).astype(np.float32)

    def coltiles(v):
        return np.ascontiguousarray(v.reshape(CT, PART).T, dtype=np.float32)

    gmat = np.zeros((PART, 8), np.float32)
    gmatt = np.zeros((8, PART), np.float32)
    for p in range(PART):
        gmat[p, p // 16] = 1.0 / 16.0
        gmatt[p // 16, p] = 1.0
    shared = {
        "mt8": mt8, "wv8": wv8, "wot": wot,
        "gw": coltiles(gn_w), "gb": coltiles(gn_b),
        "c0t": coltiles(c0), "bo2t": coltiles(bo2),
        "gmat": gmat, "gmatt": gmatt,
        "onesd": np.full((PART, 2 * PART), WSCALE, e4),
    }
    in_maps = []
    for core in range(8):
        br, hf = divmod(core, 2)
        xap = x[br].reshape(C, N)
        if hf:
            xap = np.concatenate([xap[:, NQ:], xap[:, :NQ]], axis=1)
        xap = np.ascontiguousarray(xap, dtype=np.float32)
        in_maps.append({"xa": xap, "xb": xap.astype(ml_dtypes.bfloat16), **shared})
    return in_maps


def assemble_output(results, b=4, hh=64, ww=64):
    out = np.zeros((b, C, N), np.float32)
    for core in range(8):
        br, hf = divmod(core, 2)
        out[br][:, hf * NQ:(hf + 1) * NQ] = results[core]["y"]
    return out.reshape(b, C, hh, ww)


def kernel(x, gn_w, gn_b, qkv_w, qkv_b, out_w, out_b):
    from concourse import bass_utils
    in_maps = prep_inputs(x, gn_w, gn_b, qkv_w, qkv_b, out_w, out_b)
    nc = build_program()
    res = bass_utils.run_bass_kernel_spmd(nc, in_maps, core_ids=list(range(8)))
    return assemble_output(res.results)


# revision 11
# speedup vs baseline: 1.4890x; 1.0440x over previous
"""Trainium2 Bass kernel for nn_AttentionBlock (GroupNorm + single-head
self-attention over 64x64 spatial + out-projection + residual).

Sharding: 8 cores = 4 batches x 2 query-halves. Each core receives its
batch's x as [512, 4096] (channels x pixels), rotated so that its own
2048 query pixels are columns 0:2048. GroupNorm stats / keys / values
span all 4096 pixels (invariant to the rotation), so the program is
identical on every core (pure SPMD, no collectives); the host gathers
the 8 [512, 2048] outputs back into (4, 512, 64, 64).

Algebraic restructuring (host-precomputed):
  - scores^T = h^T (A h + c0),  A = Wk^T Wq, c0 = Wk^T bq: k/q are never
    materialized (the k-bias term is constant within each softmax
    column and cancels exactly).
  - v^T = (Wv h)^T with no bias; bv commutes through the attention
    average and folds into bo2 = out_w @ bv + out_b.
  - softmax without max-subtraction; exp is biased by -ESHIFT so that
    E stays within fp8-e4m3 range (the shift cancels exactly in the
    normalization since the denominator is built from the same E).

Precision: the four large matmul families (u = A h, v = Wv h,
scores = h^T u, numerator = v E) run in fp8-e4m3 with
perf_mode=DoubleRow (two contraction elements per PE cell -> K=256 per
matmul, 2x PE throughput). A and Wv are pre-scaled by 16 to sit in
e4m3's normal range; the scales are folded into the exp scale and the
softmax reciprocal (the denominator's ones-matmul uses stationary
value 16 to absorb the v-scale for free). The out-projection stays
bf16 on the softmax-normalized attention output. GroupNorm statistics
are fp32 on a bf16 copy of x; the residual uses the original fp32 x.
Measured relative error vs the fp32 reference: ~3e-3.

The softmax denominator is accumulated on the PE (one DoubleRow
ones-matmul per key-tile-pair into a dedicated PSUM bank) instead of
DVE adds, keeping the vector engine far below the PE roofline. The
weighted-value matmuls lag the score matmuls by one key-tile-pair so
the Exp activation latency is hidden; per-chunk u-projections and the
previous chunk's out-projection are injected into the key loop so
chunk boundaries stay dense on the PE.

Infrastructure workarounds (this container's walrus accepts at most
one sync-wait per instruction): Tile's kernel-tail drain waits are
re-emitted as single-wait NOPs, and a post-scheduling pass hoists
extra waits from any instruction onto preceding single-wait NOPs.
"""

import numpy as np
import ml_dtypes

import concourse.bass as bass
import concourse.bass_isa as bass_isa
from concourse import library_config
import concourse.mybir as mybir
import concourse.tile as tile
from concourse.tile_scheduler import N_PROCS
from concourse.vector_clock import ScopedClock, VectorClock

F32 = mybir.dt.float32
F32R = mybir.dt.float32r
BT = mybir.dt.bfloat16
FP8 = mybir.dt.float8e4
AF = mybir.ActivationFunctionType
OP = mybir.AluOpType
DR = mybir.MatmulPerfMode.DoubleRow

PART = 128
C = 512          # channels
N = 4096         # pixels per batch
NQ = 2048        # query pixels per core
CT = C // PART   # 4 channel tiles
NKT = N // PART  # 32 key tiles
NTP = NKT // 2   # 16 key tile pairs
CH = 512         # nq chunk width
JCH = NQ // CH   # 4 chunks
EPS = 1e-5
SCALE = float(C) ** -0.5
WSCALE = 16.0    # fp8 pre-scale on A and Wv
ESHIFT = 2.0     # exp bias: E = exp(s*SCALE - ESHIFT), cancels in softmax


def _patched_drain_and_barrier(self, tick_clock, wait_clock):
    # Walrus in this container accepts at most one sync-wait per
    # instruction; Tile's stock exit path stacks every outstanding
    # proc's wait on a single SP Drain. Emit one single-wait NOP per
    # proc instead, then a wait-free drain.
    nc = self.nc
    gc = tick_clock.global_clock
    for p in range(N_PROCS):
        t = gc[p]
        if t <= 0:
            continue
        vc = VectorClock([t if q == p else 0 for q in range(N_PROCS)])
        nop = nc.sync.nop(nofuse=True, hint=f"drainwait{p}")
        wait_clock.add_sem_waits(nop.ins, ScopedClock({None: vc}))
    nc.sync.drain()

    nc.all_engine_barrier()
    assert self.sems is not None
    popped = nc._tile_sem_poison_stack.pop()
    assert popped is self._sem_poison
    nc.clear_and_free_semaphores(list(self.sems.allocated().values()))


def apply_tile_patch():
    tile.TileContext._drain_and_barrier = _patched_drain_and_barrier


def split_multi_waits(nc):
    """Walrus in this container accepts at most one sync-wait command per
    instruction. Tile's wait-assignment freely stacks several. Hoist all
    but the last wait of each instruction onto single-wait NOPs inserted
    immediately before it on the same engine (engine blocks on each in
    turn, so the gating is equivalent)."""
    k = 0
    for fn in nc.m.functions:
        for bb in fn.blocks:
            il = bb.instructions
            i = 0
            while i < len(il):
                inst = il[i]
                si = inst.sync_info
                waits = list(si.on_wait) if si and si.on_wait else []
                if len(waits) > 1:
                    for w in waits[:-1]:
                        nop = mybir.InstNoOp(name=f"I-waitsplit-{k}")
                        k += 1
                        nop.engine = inst.engine
                        nop.sync_info = mybir.SyncInfo(on_wait=[w], on_update=[])
                        il.insert(i, nop)
                        i += 1
                    si.on_wait = [waits[-1]]
                    inst.sync_info = si
                i += 1


def build_program(split_waits=True):
    apply_tile_patch()
    nc = bass.Bass(name="attnblk")
    xa = nc.dram_tensor("xa", [C, N], F32, kind="ExternalInput").ap()
    xb = nc.dram_tensor("xb", [C, N], BT, kind="ExternalInput").ap()
    # weights packed as [128, CT*C] so DMA rows are 2KB+ contiguous
    mt8d = nc.dram_tensor("mt8", [PART, CT * C], FP8, kind="ExternalInput").ap()
    wv8d = nc.dram_tensor("wv8", [PART, CT * C], FP8, kind="ExternalInput").ap()
    wotd = nc.dram_tensor("wot", [PART, CT * C], BT, kind="ExternalInput").ap()
    gw = nc.dram_tensor("gw", [PART, CT], F32, kind="ExternalInput").ap()
    gb = nc.dram_tensor("gb", [PART, CT], F32, kind="ExternalInput").ap()
    c0t = nc.dram_tensor("c0t", [PART, CT], F32, kind="ExternalInput").ap()
    bo2t = nc.dram_tensor("bo2t", [PART, CT], F32, kind="ExternalInput").ap()
    gmat = nc.dram_tensor("gmat", [PART, 8], F32R, kind="ExternalInput").ap()
    gmatt = nc.dram_tensor("gmatt", [8, PART], F32R, kind="ExternalInput").ap()
    onesd = nc.dram_tensor("onesd", [PART, 2 * PART], FP8, kind="ExternalInput").ap()
    y = nc.dram_tensor("y", [C, NQ], F32, kind="ExternalOutput").ap()

    with tile.TileContext(nc) as tc:
        with (
            tc.tile_pool(name="const", bufs=1) as cp,
            tc.tile_pool(name="wts", bufs=1) as wp,
            tc.tile_pool(name="hp", bufs=1) as hp,
            tc.tile_pool(name="vtp", bufs=1) as vp,
        ):
            # x tiles dominate the critical path: their descriptors go first
            # on the SP DGE queue (Phase A below). Everything else issues on
            # the ACT DGE queue and transfers in parallel.
            gwt = cp.tile([PART, CT], F32)
            gbt = cp.tile([PART, CT], F32)
            c0s = cp.tile([PART, CT], F32)
            bo2s = cp.tile([PART, CT], F32)
            gm = cp.tile([PART, 8], F32R)
            gmt = cp.tile([8, PART], F32R)
            ones8 = cp.tile([PART, 2, PART], FP8)
            epst = cp.tile([PART, 1], F32)
            nc.vector.memset(epst, EPS)
            esh = cp.tile([PART, 1], F32)
            nc.vector.memset(esh, -ESHIFT)

            mts8 = wp.tile([PART, CT, C], FP8)
            wvs8 = wp.tile([PART, CT, C], FP8)
            wots = wp.tile([PART, CT, C], BT)

            # ACT-queue weight DMAs staged between per-tile stats blocks so
            # descriptor issue doesn't delay the stats activations
            weight_dma_stages = [
                lambda: (nc.scalar.dma_start(out=gwt, in_=gw),
                         nc.scalar.dma_start(out=gbt, in_=gb),
                         nc.scalar.dma_start(out=gm, in_=gmat),
                         nc.scalar.dma_start(out=gmt, in_=gmatt)),
                lambda: (nc.scalar.dma_start(out=wvs8, in_=wv8d),
                         nc.scalar.dma_start(out=mts8, in_=mt8d),
                         nc.scalar.dma_start(out=c0s, in_=c0t)),
                lambda: (nc.scalar.dma_start(out=ones8, in_=onesd),
                         nc.scalar.dma_start(out=bo2s, in_=bo2t)),
                lambda: (nc.scalar.dma_start(out=wots, in_=wotd),),
            ]

            h8 = hp.tile([PART, CT, N], FP8, name="h8")
            v8 = vp.tile([PART, NTP, 2, CH], FP8, name="v8")

            # ---- Phase A: GroupNorm stats + normalize into fp8 h ----
            with (
                tc.tile_pool(name="stats", bufs=2) as sp,
                tc.tile_pool(name="coef", bufs=1) as cfp,
                tc.tile_pool(name="xst", bufs=4) as xp,
                tc.tile_pool(name="pst", bufs=2, space="PSUM") as pp,
            ):
                acoef = cfp.tile([PART, CT], F32)
                bcoef = cfp.tile([PART, CT], F32)

                def stats_halves(xt, s2):
                    # DVE bn_stats on the first half, ACT sum/sumsq
                    # (accum_out) on the second half, combined into
                    # (mean, E[x^2]) per channel
                    xr = xt[:, 0:N // 2].rearrange("p (s f) -> p s f", f=512)
                    st6 = sp.tile([PART, 4, 6], F32, tag="st6")
                    for s in range(4):
                        nc.vector.bn_stats(out=st6[:, s, :], in_=xr[:, s, :])
                    mv = sp.tile([PART, 2], F32, tag="mv")
                    nc.vector.bn_aggr(out=mv, in_=st6)
                    ssc = sp.tile([PART, N // 2], BT, tag="ssc")
                    asum = sp.tile([PART, 1], F32, tag="asum")
                    asq = sp.tile([PART, 1], F32, tag="asq")
                    nc.scalar.activation(out=ssc, in_=xt[:, N // 2:],
                                         func=AF.Identity, accum_out=asum)
                    ssc2 = sp.tile([PART, N // 2], BT, tag="ssc")
                    nc.scalar.activation(out=ssc2, in_=xt[:, N // 2:],
                                         func=AF.Square, accum_out=asq)
                    # mean = mean_a/2 + sum_b/N
                    tmA = sp.tile([PART, 1], F32, tag="tmA")
                    nc.vector.tensor_scalar(
                        out=tmA, in0=asum, scalar1=1.0 / N, scalar2=None,
                        op0=OP.mult)
                    nc.vector.tensor_scalar(
                        out=s2[:, 0:1], in0=mv[:, 0:1], scalar1=0.5,
                        scalar2=None, op0=OP.mult)
                    nc.vector.tensor_add(out=s2[:, 0:1], in0=s2[:, 0:1], in1=tmA)
                    # E[x^2] = (var_a + mean_a^2)/2 + sumsq_b/N
                    tmB = sp.tile([PART, 1], F32, tag="tmB")
                    nc.vector.tensor_tensor(
                        out=tmB, in0=mv[:, 0:1], in1=mv[:, 0:1], op=OP.mult)
                    nc.vector.tensor_add(out=tmB, in0=tmB, in1=mv[:, 1:2])
                    tmC = sp.tile([PART, 1], F32, tag="tmC")
                    nc.vector.tensor_scalar(
                        out=tmC, in0=asq, scalar1=1.0 / N, scalar2=None,
                        op0=OP.mult)
                    nc.vector.tensor_scalar(
                        out=tmB, in0=tmB, scalar1=0.5, scalar2=None,
                        op0=OP.mult)
                    nc.vector.tensor_add(out=s2[:, 1:2], in0=tmB, in1=tmC)

                for ci in range(CT):
                    xt = xp.tile([PART, N], BT, tag="x")
                    nc.sync.dma_start(
                        out=xt[:, 0:N // 2],
                        in_=xb[ci * PART:(ci + 1) * PART, 0:N // 2])
                    nc.sync.dma_start(
                        out=xt[:, N // 2:],
                        in_=xb[ci * PART:(ci + 1) * PART, N // 2:])
                    s2 = sp.tile([PART, 2], F32R, tag="s2")
                    stats_halves(xt, s2)
                    weight_dma_stages[ci]()
                    # group means over 16-channel blocks: [8, 2]
                    gp_ = pp.tile([8, 2], F32, tag="gp")
                    nc.tensor.matmul(gp_, lhsT=gm, rhs=s2,
                                     start=True, stop=True)
                    gs = sp.tile([8, 2], F32R, tag="gs")
                    nc.vector.tensor_copy(out=gs, in_=gp_)
                    msq = sp.tile([8, 1], F32, tag="msq")
                    nc.vector.tensor_tensor(
                        out=msq, in0=gs[:, 0:1], in1=gs[:, 0:1], op=OP.mult)
                    nc.vector.tensor_sub(out=gs[:, 1:2], in0=gs[:, 1:2], in1=msq)
                    nc.scalar.activation(out=gs[:, 1:2], in_=gs[:, 1:2],
                                         func=AF.Sqrt, bias=epst[0:8])
                    with nc.allow_low_precision(
                            reason="fp32r rounding for PE broadcast matmul"):
                        nc.vector.reciprocal(out=gs[:, 1:2], in_=gs[:, 1:2])
                    # broadcast per-group (mean, rstd) back to channels
                    cb = pp.tile([PART, 2], F32, tag="cb")
                    nc.tensor.matmul(cb, lhsT=gmt, rhs=gs,
                                     start=True, stop=True)
                    nc.vector.tensor_tensor(
                        out=acoef[:, ci:ci + 1], in0=cb[:, 1:2],
                        in1=gwt[:, ci:ci + 1], op=OP.mult)
                    tmpb = sp.tile([PART, 1], F32, tag="tmpb")
                    nc.vector.tensor_tensor(
                        out=tmpb, in0=cb[:, 0:1], in1=acoef[:, ci:ci + 1], op=OP.mult)
                    nc.vector.tensor_sub(
                        out=bcoef[:, ci:ci + 1], in0=gbt[:, ci:ci + 1], in1=tmpb)
                    # h = x * a + b, straight to fp8
                    nc.vector.tensor_scalar(
                        out=h8[:, ci, :], in0=xt,
                        scalar1=acoef[:, ci:ci + 1], scalar2=bcoef[:, ci:ci + 1],
                        op0=OP.mult, op1=OP.add)

            # ---- Phase C: v^T tiles (fp8 DoubleRow) ----
            with tc.tile_pool(name="vps", bufs=4, space="PSUM") as vpp:
                for t in range(NKT):
                    vps = vpp.tile([PART, CH], F32, tag="vps")
                    ksl = slice(t * PART, (t + 1) * PART)
                    for a in range(2):
                        nc.tensor.matmul(
                            vps,
                            lhsT=h8[:, 2 * a:2 * a + 2, ksl],
                            rhs=wvs8[:, 2 * a:2 * a + 2, :],
                            start=(a == 0), stop=(a == 1), perf_mode=DR)
                    nc.vector.tensor_copy(out=v8[:, t // 2, t % 2, :], in_=vps)

            # ---- Phase D+E: attention + out-projection, per nq-chunk ----
            with (
                tc.tile_pool(name="ujp", bufs=2) as up,
                tc.tile_pool(name="ep", bufs=3) as ep,
                tc.tile_pool(name="attp", bufs=2) as ap_,
                tc.tile_pool(name="rcp", bufs=2) as rp,
                tc.tile_pool(name="xrp", bufs=8) as xrp,
                tc.tile_pool(name="otp", bufs=2) as otp,
                tc.tile_pool(name="oup", bufs=1, space="PSUM") as oup,
                tc.tile_pool(name="stp", bufs=2, space="PSUM") as stp,
                tc.tile_pool(name="ddp", bufs=1, space="PSUM") as ddp,
                tc.tile_pool(name="fpp", bufs=1, space="PSUM") as fpp,
            ):
                u8s = {}
                ous = {}
                dds = {}
                atts = {}
                xrbs = {}

                def emit_u(jc):
                    # u_jc = A16 h[:, chunk jc] + 16*c0, fp8
                    ut = up.tile([PART, CT, CH], FP8, tag="uj", name=f"uj{jc}")
                    sl = slice(jc * CH, (jc + 1) * CH)
                    for i in range(CT):
                        ups = fpp.tile([PART, CH], F32, tag="fpu")
                        for a in range(2):
                            nc.tensor.matmul(
                                ups,
                                lhsT=mts8[:, 2 * a:2 * a + 2, i * PART:(i + 1) * PART],
                                rhs=h8[:, 2 * a:2 * a + 2, sl],
                                start=(a == 0), stop=(a == 1), perf_mode=DR)
                        nc.vector.tensor_scalar(
                            out=ut[:, i, :], in0=ups,
                            scalar1=c0s[:, i:i + 1], scalar2=None, op0=OP.add)
                    u8s[jc] = ut

                def emit_ou(j, et, tp):
                    if tp == 0:
                        ous[j] = [oup.tile([PART, CH], F32, tag=f"ou{m}",
                                           name=f"ou{m}_{j}") for m in range(CT)]
                        dds[j] = ddp.tile([PART, CH], F32, tag="dd", name=f"dd{j}")
                    for m in range(CT):
                        nc.tensor.matmul(
                            ous[j][m],
                            lhsT=v8[:, tp, :, m * PART:(m + 1) * PART],
                            rhs=et,
                            start=(tp == 0), stop=(tp == NTP - 1), perf_mode=DR)
                    nc.tensor.matmul(
                        dds[j], lhsT=ones8, rhs=et,
                        start=(tp == 0), stop=(tp == NTP - 1), perf_mode=DR)

                def emit_xrb(j, m):
                    jsl = slice(j * CH, (j + 1) * CH)
                    xr_ = xrp.tile([PART, CH], F32, tag="xr")
                    nc.sync.dma_start(out=xr_, in_=xa[m * PART:(m + 1) * PART, jsl])
                    xrb = xrp.tile([PART, CH], F32, tag="xrb")
                    nc.vector.tensor_scalar(
                        out=xrb, in0=xr_, scalar1=bo2s[:, m:m + 1],
                        scalar2=None, op0=OP.add)
                    xrbs[(j, m)] = xrb

                rcs = {}

                def emit_rc_att(j):
                    rc = rp.tile([PART, CH], F32, tag="rc")
                    nc.vector.reciprocal(out=rc, in_=dds[j])
                    rcs[j] = rc
                    if j < JCH - 1:
                        # normalize during the PSUM->SBUF copy; the out-proj
                        # result then needs only the residual add
                        att = ap_.tile([PART, CT, CH], BT, tag="att", name=f"att{j}")
                        for m in range(CT):
                            nc.vector.tensor_tensor(
                                out=att[:, m, :], in0=ous[j][m], in1=rc, op=OP.mult)
                        atts[j] = att

                def emit_att_raw(j):
                    # last chunk: copy unnormalized so the out-proj matmuls
                    # don't wait on the reciprocal; rc folds into the epilogue
                    att = ap_.tile([PART, CT, CH], BT, tag="att", name=f"att{j}")
                    for m in range(CT):
                        if m % 2 == 0:
                            nc.vector.tensor_copy(out=att[:, m, :], in_=ous[j][m])
                        else:
                            nc.scalar.copy(out=att[:, m, :], in_=ous[j][m])
                    atts[j] = att

                def emit_fp(j, m, pool, tag, normalized=True):
                    # out-projection tile m of chunk j (bf16) + bias/residual + store
                    jsl = slice(j * CH, (j + 1) * CH)
                    fp = pool.tile([PART, CH], F32, tag=tag)
                    for ci in range(CT):
                        nc.tensor.matmul(
                            fp,
                            lhsT=wots[:, ci, m * PART:(m + 1) * PART],
                            rhs=atts[j][:, ci, :],
                            start=(ci == 0), stop=(ci == CT - 1))
                    ot = otp.tile([PART, CH], F32, tag="ot")
                    if normalized:
                        nc.vector.tensor_add(out=ot, in0=fp, in1=xrbs[(j, m)])
                    else:
                        nc.vector.tensor_tensor(out=ot, in0=fp, in1=rcs[j], op=OP.mult)
                        nc.vector.tensor_add(out=ot, in0=ot, in1=xrbs[(j, m)])
                    nc.sync.dma_start(out=y[m * PART:(m + 1) * PART, jsl], in_=ot)

                emit_u(0)
                for j in range(JCH):
                    uj = u8s[j]
                    prev_et = None
                    for tp in range(NTP):
                        st_ = []
                        for half in range(2):
                            t = 2 * tp + half
                            ksl = slice(t * PART, (t + 1) * PART)
                            st = stp.tile([PART, CH], F32, tag="st")
                            for a in range(2):
                                nc.tensor.matmul(
                                    st,
                                    lhsT=h8[:, 2 * a:2 * a + 2, ksl],
                                    rhs=uj[:, 2 * a:2 * a + 2, :],
                                    start=(a == 0), stop=(a == 1), perf_mode=DR)
                            st_.append(st)
                        if prev_et is not None:
                            emit_ou(j, prev_et, tp - 1)
                        if j > 0 and 1 <= tp < 1 + CT:
                            emit_fp(j - 1, tp - 1, fpp, "fpu")
                        if CT < tp <= 2 * CT:
                            emit_xrb(j, tp - CT - 1)
                        et = ep.tile([PART, 2, CH], FP8, tag="et")
                        for half in range(2):
                            nc.scalar.activation(
                                out=et[:, half, :], in_=st_[half],
                                func=AF.Exp, scale=SCALE / WSCALE, bias=esh)
                        prev_et = et
                        if tp == NTP - 3 and j + 1 < JCH:
                            emit_u(j + 1)
                    emit_ou(j, prev_et, NTP - 1)
                    if j < JCH - 1:
                        emit_rc_att(j)
                # final chunk: unnormalized copies + rc off the critical path
                emit_att_raw(JCH - 1)
                emit_rc_att(JCH - 1)
                for m in range(CT):
                    emit_fp(JCH - 1, m, stp, "st", normalized=False)
    if split_waits:
        split_multi_waits(nc)
    return nc


def prep_inputs(x, gn_w, gn_b, qkv_w, qkv_b, out_w, out_b):
    x = np.asarray(x, np.float32)
    gn_w = np.asarray(gn_w, np.float32)
    gn_b = np.asarray(gn_b, np.float32)
    qkv_w = np.asarray(qkv_w, np.float32)
    qkv_b = np.asarray(qkv_b, np.float32)
    out_w = np.asarray(out_w, np.float32)
    out_b = np.asarray(out_b, np.float32)

    Wq, Wk, Wv = qkv_w[0:C], qkv_w[C:2 * C], qkv_w[2 * C:3 * C]
    bq, bv = qkv_b[0:C], qkv_b[2 * C:3 * C]
    bf16 = ml_dtypes.bfloat16
    e4 = ml_dtypes.float8_e4m3

    def packrows(w):
        # [C, C] -> [PART, CT*C] so SBUF tile [PART, CT, C] loads in one
        # wide-row DMA: packed[p, j*C+col] = w[j*PART+p, col]
        return np.ascontiguousarray(
            w.reshape(CT, PART, C).transpose(1, 0, 2).reshape(PART, CT * C))

    mt8 = packrows((WSCALE * (Wq.T @ Wk)).astype(e4))
    wv8 = packrows((WSCALE * Wv.T).astype(e4))
    wot = packrows(out_w.T.astype(bf16))
    c0 = (WSCALE * (Wk.T @ bq)).astype(np.float32)
    bo2 = (out_w @ bv + out_b).astype(np.float32)

    def coltiles(v):
        return np.ascontiguousarray(v.reshape(CT, PART).T, dtype=np.float32)

    gmat = np.zeros((PART, 8), np.float32)
    gmatt = np.zeros((8, PART), np.float32)
    for p in range(PART):
        gmat[p, p // 16] = 1.0 / 16.0
        gmatt[p // 16, p] = 1.0
    shared = {
        "mt8": mt8, "wv8": wv8, "wot": wot,
        "gw": coltiles(gn_w), "gb": coltiles(gn_b),
        "c0t": coltiles(c0), "bo2t": coltiles(bo2),
        "gmat": gmat, "gmatt": gmatt,
        "onesd": np.full((PART, 2 * PART), WSCALE, e4),
    }
    in_maps = []
    for core in range(8):
        br, hf = divmod(core, 2)
        xap = x[br].reshape(C, N)
        if hf:
            xap = np.concatenate([xap[:, NQ:], xap[:, :NQ]], axis=1)
        xap = np.ascontiguousarray(xap, dtype=np.float32)
        in_maps.append({"xa": xap, "xb": xap.astype(ml_dtypes.bfloat16), **shared})
    return in_maps


def assemble_output(results, b=4, hh=64, ww=64):
    out = np.zeros((b, C, N), np.float32)
    for core in range(8):
        br, hf = divmod(core, 2)
        out[br][:, hf * NQ:(hf + 1) * NQ] = results[core]["y"]
    return out.reshape(b, C, hh, ww)


def kernel(x, gn_w, gn_b, qkv_w, qkv_b, out_w, out_b):
    from concourse import bass_utils
    in_maps = prep_inputs(x, gn_w, gn_b, qkv_w, qkv_b, out_w, out_b)
    nc = build_program()
    res = bass_utils.run_bass_kernel_spmd(nc, in_maps, core_ids=list(range(8)))
    return assemble_output(res.results)
